# revision 13
# baseline (speedup 1.0000x reference)
"""Trainium2 Bass kernel for nn_CNV_Cifar10_Binary (binary CNN, CIFAR-like).

Strategy (pure data parallel, 8 cores x 64 images):
 - All binarized convs/FCs run as bf16 matmuls (values are exactly +-1 -> exact).
 - Layer 0 (real-valued input) runs in fp32 with 4-way PE row-tiling (K=27 via
   host-side im2col; partition groups at base 0/32/64/96).
 - sign(BN(x)) == sign(x + c) since inv>0; applied by ScalarE Sign activation
   with per-channel bias straight out of PSUM; maxpool commutes with sign and
   is done after sign by VectorE max ops on bf16.
 - Cin=64 layers (L1, L2) pack ky in the contraction dim: partitions 64..127
   hold the same activations shifted by one row (built with one SBUF->SBUF DMA),
   so taps (ky=0,1) fuse into K=128 matmuls; ky=2 uses a K=64 matmul.
 - Final FC3 output (10 logits) is copied out raw; affine-free BN + log_softmax
   run on host (exact, negligible work).
"""

import os
import sys

import numpy as np

sys.path.insert(0, "/opt/trn_rl_repo")

import concourse.bass as bass  # noqa: E402
import concourse.bacc as bacc  # noqa: E402
import concourse.mybir as mybir  # noqa: E402
import concourse.tile as tile  # noqa: E402

F32 = mybir.dt.float32
BF16 = mybir.dt.bfloat16
AF = mybir.ActivationFunctionType
ALU = mybir.AluOpType
BF16_NP = mybir.dt.np(BF16)

N_CORES = 8
EPS = 1e-5


def build_program(N):
    """Build the per-core Bass program for N images."""
    assert N % 8 == 0
    G = 4                      # PE row-tile groups for layer 0
    npg = N // G               # images per group
    qsz = max(1, npg // 4)     # images per group per X-load block
    nq = npg // qsz            # number of X-load blocks ("quarters")
    assert nq % 2 == 0
    qph = nq // 2              # quarters per half
    NH = N // 2                # images per half (S1 buffer covers a half)

    nc = bacc.Bacc("TRN2", target_bir_lowering=False, debug=False)

    # ---- DRAM I/O ----
    xcol = nc.dram_tensor("xcol", [nq, G, 27, qsz, 35, 36], F32, kind="ExternalInput")
    w0d = nc.dram_tensor("w0", [27, 64], F32, kind="ExternalInput")
    w1ad = nc.dram_tensor("w1a", [128, 3, 64], BF16, kind="ExternalInput")
    w1bd = nc.dram_tensor("w1b", [64, 3, 64], BF16, kind="ExternalInput")
    w2ad = nc.dram_tensor("w2a", [128, 3, 128], BF16, kind="ExternalInput")
    w2bd = nc.dram_tensor("w2b", [64, 3, 128], BF16, kind="ExternalInput")
    w3d = nc.dram_tensor("w3", [128, 9, 128], BF16, kind="ExternalInput")
    w4d = nc.dram_tensor("w4", [128, 2, 9, 128], BF16, kind="ExternalInput")
    w5d = nc.dram_tensor("w5", [2, 128, 2, 9, 128], BF16, kind="ExternalInput")
    fw1d = nc.dram_tensor("fw1", [128, 2, 512], BF16, kind="ExternalInput")
    fw2d = nc.dram_tensor("fw2", [128, 4, 512], BF16, kind="ExternalInput")
    fw3d = nc.dram_tensor("fw3", [128, 4, 10], BF16, kind="ExternalInput")
    bbd = nc.dram_tensor("bb", [128, 16], F32, kind="ExternalInput")
    outd = nc.dram_tensor("out", [10, N], F32, kind="ExternalOutput")

    with tile.TileContext(nc) as tc:
        with (
            tc.tile_pool(name="wpool", bufs=1) as wp,
            tc.tile_pool(name="xpool", bufs=2) as xp,
            tc.tile_pool(name="apool", bufs=1) as ap,
            tc.tile_pool(name="spool", bufs=3) as sp,
            tc.tile_pool(name="pspool", bufs=2, space="PSUM") as pp,
            tc.tile_pool(name="pspool2", bufs=3, space="PSUM") as pp2,
            tc.tile_pool(name="dpool", bufs=1, space="DRAM") as dp,
        ):
            # ---- weights / consts ----
            W0 = wp.tile([128, 64], F32, tag="w0")
            for g in range(G):
                nc.sync.dma_start(W0[32 * g : 32 * g + 27, :], w0d[:, :])
            W1A = wp.tile([128, 3, 64], BF16, tag="w1a")
            nc.sync.dma_start(W1A[:, :, :], w1ad[:, :, :])
            W1B = wp.tile([64, 3, 64], BF16, tag="w1b")
            nc.sync.dma_start(W1B[:, :, :], w1bd[:, :, :])
            W2A = wp.tile([128, 3, 128], BF16, tag="w2a")
            nc.sync.dma_start(W2A[:, :, :], w2ad[:, :, :])
            W2B = wp.tile([64, 3, 128], BF16, tag="w2b")
            nc.sync.dma_start(W2B[:, :, :], w2bd[:, :, :])
            W3 = wp.tile([128, 9, 128], BF16, tag="w3")
            nc.sync.dma_start(W3[:, :, :], w3d[:, :, :])
            W4 = wp.tile([128, 2, 9, 128], BF16, tag="w4")
            nc.sync.dma_start(W4[:, :, :, :], w4d[:, :, :, :])
            W5 = [wp.tile([128, 2, 9, 128], BF16, tag=f"w5{i}", name=f"W5_{i}") for i in range(2)]
            for i in range(2):
                nc.sync.dma_start(W5[i][:, :, :, :], w5d[i, :, :, :, :])
            FW1 = wp.tile([128, 2, 512], BF16, tag="fw1")
            nc.sync.dma_start(FW1[:, :, :], fw1d[:, :, :])
            FW2 = wp.tile([128, 4, 512], BF16, tag="fw2")
            nc.sync.dma_start(FW2[:, :, :], fw2d[:, :, :])
            FW3 = wp.tile([128, 4, 10], BF16, tag="fw3")
            nc.sync.dma_start(FW3[:, :, :], fw3d[:, :, :])
            BB = wp.tile([128, 16], F32, tag="bb")
            nc.sync.dma_start(BB[:, :], bbd[:, :])

            # ---- persistent activation buffers ----
            S2 = ap.tile([128, N, 16, 16], BF16, tag="s2")
            S2pre = ap.tile([128, N // 2, 16, 16], BF16, tag="s2p")
            S4 = ap.tile([128, N, 6, 6], BF16, tag="s4")
            S5 = [ap.tile([128, N, 4, 4], BF16, tag=f"s5{i}", name=f"S5_{i}") for i in range(2)]
            S6 = [ap.tile([128, N], BF16, tag=f"s6{i}", name=f"S6_{i}") for i in range(2)]
            F1 = [ap.tile([128, N], BF16, tag=f"f1{i}", name=f"F1_{i}") for i in range(4)]
            F2 = [ap.tile([128, N], BF16, tag=f"f2{i}", name=f"F2_{i}") for i in range(4)]
            OB = ap.tile([10, N], F32, tag="ob")

            # ================= L0 + L1 (blocked in image halves) ==========
            # L0: 4-way PE row tiling (K=27) x 2-way col tiling. Column group 1
            # recomputes the same rows shifted +1 so PSUM partitions 64..127
            # hold the ky=1-shifted copy -> a single 128-lane Sign evac writes
            # both the activations and their shifted duplicate into S1.
            D2 = dp.tile([128, N // 2, 16, 16], BF16, tag="d2")
            for h in range(2):
                S1 = ap.tile([128, NH, 34, 34], BF16, tag="s1", name=f"S1_{h}")
                for qq in range(qph):
                    q = h * qph + qq
                    X = xp.tile([128, qsz, 35, 36], F32, tag="x", name=f"X_{q}")
                    for g in range(G):
                        nc.sync.dma_start(
                            X[32 * g : 32 * g + 27, :, :, :], xcol[q, g, :, :, :, :]
                        )
                    for k in range(qsz):
                        for g in range(G):
                            n_img = q * G * qsz + g * qsz + k
                            nl = n_img - h * NH
                            W0g = W0[32 * g : 32 * g + 27, :]
                            Xg = X[32 * g : 32 * g + 27, k]
                            P = pp2.tile([128, 2, 512], F32, tag="ps2")
                            for c, y0 in enumerate((0, 15)):
                                nc.tensor.matmul(
                                    P[0:64, c, 0:510], W0g, Xg[:, y0 : y0 + 15, 0:34],
                                    start=True, stop=True, skip_group_check=True, tile_position=(32 * g, 0))
                                nc.tensor.matmul(
                                    P[64:128, c, 0:510], W0g, Xg[:, y0 + 1 : y0 + 16, 0:34],
                                    start=True, stop=True, skip_group_check=True, tile_position=(32 * g, 64))
                            nc.scalar.activation(
                                S1[:, nl, 0:30, :], P[:, :, 0:510],
                                AF.Sign, bias=BB[:, 0:1])
                            Pt = pp.tile([128, 512], F32, tag="ps")
                            nc.tensor.matmul(
                                Pt[0:64, 0:136], W0g, Xg[:, 30:34, 0:34],
                                start=True, stop=True, skip_group_check=True, tile_position=(32 * g, 0))
                            nc.tensor.matmul(
                                Pt[64:128, 0:136], W0g, Xg[:, 31:35, 0:34],
                                start=True, stop=True, skip_group_check=True, tile_position=(32 * g, 64))
                            nc.scalar.activation(
                                S1[:, nl, 30:34, :], Pt[:, 0:136],
                                AF.Sign, bias=BB[:, 0:1])

                # ---- L1: 64->64, 32x32 out, pool to 16x16 ----
                # col-tile over image pairs: psum parts 0..63 <- img nA,
                # parts 64..127 <- img nB; 128-lane sign + pool into S2pre.
                for nl in range(NH // 2):
                    nA, nB = nl, nl + NH // 2
                    pidx = h * (NH // 2) + nl
                    for y0 in (0, 16):
                        P = pp.tile([128, 512], F32, tag="ps")
                        for kx in range(3):
                            nc.tensor.matmul(
                                P[0:64, :], W1A[:, kx, :],
                                S1[0:128, nA, y0 : y0 + 16, kx : kx + 32],
                                start=(kx == 0), stop=False, skip_group_check=True, tile_position=(0, 0))
                            nc.tensor.matmul(
                                P[64:128, :], W1A[:, kx, :],
                                S1[0:128, nB, y0 : y0 + 16, kx : kx + 32],
                                start=(kx == 0), stop=False, skip_group_check=True, tile_position=(0, 64))
                        for kx in range(3):
                            nc.tensor.matmul(
                                P[0:64, :], W1B[0:64, kx, :],
                                S1[0:64, nA, y0 + 2 : y0 + 18, kx : kx + 32],
                                start=False, stop=(kx == 2), skip_group_check=True, tile_position=(0, 0))
                            nc.tensor.matmul(
                                P[64:128, :], W1B[0:64, kx, :],
                                S1[0:64, nB, y0 + 2 : y0 + 18, kx : kx + 32],
                                start=False, stop=(kx == 2), skip_group_check=True, tile_position=(0, 64))
                        T = sp.tile([128, 16, 16, 2], BF16, tag="t1")
                        nc.scalar.activation(
                            T[:, :, :, :], P[:, :], AF.Sign, bias=BB[:, 1:2]
                        )
                        U = sp.tile([128, 8, 2, 16], BF16, tag="u1")
                        nc.vector.tensor_tensor(
                            U[:, :, :, :], T[:, :, :, 0], T[:, :, :, 1], op=ALU.max
                        )
                        nc.vector.tensor_tensor(
                            S2pre[:, pidx, y0 // 2 : y0 // 2 + 8, :],
                            U[:, :, 0, :], U[:, :, 1, :], op=ALU.max,
                        )
                nc.sync.dma_start(
                    D2[:, h * (NH // 2) : (h + 1) * (NH // 2)],
                    S2pre[:, h * (NH // 2) : (h + 1) * (NH // 2)],
                )
            # re-layout S2pre (img-pair partitions) -> S2 (ky-dup partitions)
            # via DRAM bounce (SBUF->SBUF DMA unsupported).
            NH2 = N // 4
            for h in range(2):
                pa, pb = NH2 * 2 * h, NH2 * 2 * h + NH2
                nc.sync.dma_start(S2[0:64, pa : pa + NH2], D2[0:64, h * NH2 : (h + 1) * NH2])
                nc.sync.dma_start(S2[0:64, pb : pb + NH2], D2[64:128, h * NH2 : (h + 1) * NH2])
                nc.sync.dma_start(S2[64:128, pa : pa + NH2, 0:15, :],
                                  D2[0:64, h * NH2 : (h + 1) * NH2, 1:16, :])
                nc.sync.dma_start(S2[64:128, pb : pb + NH2, 0:15, :],
                                  D2[64:128, h * NH2 : (h + 1) * NH2, 1:16, :])
            # ================= L2: 64->128, 16x16 -> 14x14 ================
            S3 = ap.tile([128, N, 14, 14], BF16, tag="s1")  # reuse S1 slot
            for n0 in range(0, N, 2):
                P = pp.tile([128, 2, 14, 14], F32, tag="ps")
                for kx in range(3):
                    nc.tensor.matmul(
                        P[:, :, :, :],
                        W2A[:, kx, :],
                        S2[0:128, n0 : n0 + 2, 0:14, kx : kx + 14],
                        start=(kx == 0),
                        stop=False,
                    )
                for kx in range(3):
                    nc.tensor.matmul(
                        P[:, :, :, :],
                        W2B[0:64, kx, :],
                        S2[0:64, n0 : n0 + 2, 2:16, kx : kx + 14],
                        start=False,
                        stop=(kx == 2),
                    )
                nc.scalar.activation(
                    S3[:, n0 : n0 + 2, :, :], P[:, :, :, :], AF.Sign, bias=BB[:, 2:3]
                )

            # ================= L3: 128->128, 14->12, pool to 6x6 ==========
            n0 = 0
            while n0 < N:
                nn = min(3, N - n0)
                P = pp.tile([128, 3, 12, 12], F32, tag="ps")
                t = 0
                for ky in range(3):
                    for kx in range(3):
                        nc.tensor.matmul(
                            P[:, :nn, :, :],
                            W3[:, t, :],
                            S3[:, n0 : n0 + nn, ky : ky + 12, kx : kx + 12],
                            start=(t == 0),
                            stop=(t == 8),
                        )
                        t += 1
                T = sp.tile([128, 3, 12, 6, 2], BF16, tag="t3")
                nc.scalar.activation(
                    T[:, :nn, :, :, :], P[:, :nn, :, :], AF.Sign, bias=BB[:, 3:4]
                )
                U = sp.tile([128, 3, 6, 2, 6], BF16, tag="u3")
                nc.vector.tensor_tensor(
                    U[:, :nn, :, :, :], T[:, :nn, :, :, 0], T[:, :nn, :, :, 1], op=ALU.max
                )
                nc.vector.tensor_tensor(
                    S4[:, n0 : n0 + nn, :, :], U[:, :nn, :, 0, :], U[:, :nn, :, 1, :],
                    op=ALU.max,
                )
                n0 += nn

            # ================= L4: 128->256, 6->4 =========================
            for n0 in range(0, N, 32):
                nn = min(32, N - n0)
                for cg in range(2):
                    P = pp.tile([128, 32, 4, 4], F32, tag="ps")
                    t = 0
                    for ky in range(3):
                        for kx in range(3):
                            nc.tensor.matmul(
                                P[:, :nn, :, :],
                                W4[:, cg, t, :],
                                S4[:, n0 : n0 + nn, ky : ky + 4, kx : kx + 4],
                                start=(t == 0),
                                stop=(t == 8),
                            )
                            t += 1
                    nc.scalar.activation(
                        S5[cg][:, n0 : n0 + nn, :, :],
                        P[:, :nn, :, :],
                        AF.Sign,
                        bias=BB[:, 4 + cg : 5 + cg],
                    )

            # ================= L5: 256->256, 4->2, pool to 1 ==============
            for cg in range(2):
                P = pp.tile([128, N, 2, 2], F32, tag="ps")
                first = True
                for ci in range(2):
                    t = 0
                    for ky in range(3):
                        for kx in range(3):
                            nc.tensor.matmul(
                                P[:, :, :, :],
                                W5[ci][:, cg, t, :],
                                S5[ci][:, :, ky : ky + 2, kx : kx + 2],
                                start=first,
                                stop=(ci == 1 and t == 8),
                            )
                            first = False
                            t += 1
                T = sp.tile([128, N, 2, 2], BF16, tag="t5")
                nc.scalar.activation(
                    T[:, :, :, :], P[:, :, :, :], AF.Sign, bias=BB[:, 6 + cg : 7 + cg]
                )
                U = sp.tile([128, N, 2], BF16, tag="u5")
                nc.vector.tensor_tensor(U[:, :, :], T[:, :, :, 0], T[:, :, :, 1], op=ALU.max)
                nc.vector.tensor_tensor(S6[cg][:, :], U[:, :, 0], U[:, :, 1], op=ALU.max)

            # ================= FC1/FC2/FC3 ================================
            for g in range(4):
                P = pp.tile([128, N], F32, tag="ps")
                for kc in range(2):
                    nc.tensor.matmul(
                        P[:, :], FW1[:, kc, 128 * g : 128 * g + 128], S6[kc][:, :],
                        start=(kc == 0), stop=(kc == 1),
                    )
                nc.scalar.activation(F1[g][:, :], P[:, :], AF.Sign, bias=BB[:, 8 + g : 9 + g])
            for g in range(4):
                P = pp.tile([128, N], F32, tag="ps")
                for kc in range(4):
                    nc.tensor.matmul(
                        P[:, :], FW2[:, kc, 128 * g : 128 * g + 128], F1[kc][:, :],
                        start=(kc == 0), stop=(kc == 3),
                    )
                nc.scalar.activation(F2[g][:, :], P[:, :], AF.Sign, bias=BB[:, 12 + g : 13 + g])
            P = pp.tile([10, N], F32, tag="ps")
            for kc in range(4):
                nc.tensor.matmul(
                    P[:, :], FW3[:, kc, :], F2[kc][:, :],
                    start=(kc == 0), stop=(kc == 3),
                )
            nc.vector.tensor_copy(OB[:, :], P[:, :])
            nc.sync.dma_start(outd[:, :], OB[:, :])

    return nc


def prep_shared(conv_ws, conv_gammas, conv_betas, conv_means, conv_vars,
                fc_ws, fc_gammas, fc_betas, fc_means, fc_vars):
    """Host-side: fold BN into per-channel sign thresholds; lay out weights."""
    s = [np.sign(np.asarray(w, np.float32)) for w in conv_ws]
    sf = [np.sign(np.asarray(w, np.float32)) for w in fc_ws]

    def thr(g, b, m, v):
        inv = np.asarray(g) / np.sqrt(np.asarray(v) + EPS)
        assert np.all(inv > 0)
        return (np.asarray(b) / inv - np.asarray(m)).astype(np.float32)

    d = {}
    d["w0"] = s[0].transpose(2, 3, 1, 0).reshape(27, 64).astype(np.float32)
    for i, nm in ((1, "w1"), (2, "w2")):
        a = s[i].transpose(2, 1, 3, 0)  # (ky, ci, kx, co)
        d[nm + "a"] = np.ascontiguousarray(a[0:2].reshape(128, 3, -1)).astype(BF16_NP)
        d[nm + "b"] = np.ascontiguousarray(a[2]).astype(BF16_NP)
    d["w3"] = np.ascontiguousarray(
        s[3].transpose(1, 2, 3, 0).reshape(128, 9, 128)
    ).astype(BF16_NP)
    d["w4"] = np.ascontiguousarray(
        s[4].reshape(2, 128, 128, 3, 3).transpose(2, 0, 3, 4, 1).reshape(128, 2, 9, 128)
    ).astype(BF16_NP)
    d["w5"] = np.ascontiguousarray(
        s[5].reshape(2, 128, 2, 128, 3, 3).transpose(2, 3, 0, 4, 5, 1)
        .reshape(2, 128, 2, 9, 128)
    ).astype(BF16_NP)
    d["fw1"] = np.ascontiguousarray(
        sf[0].T.reshape(2, 128, 512).transpose(1, 0, 2)
    ).astype(BF16_NP)
    d["fw2"] = np.ascontiguousarray(
        sf[1].T.reshape(4, 128, 512).transpose(1, 0, 2)
    ).astype(BF16_NP)
    d["fw3"] = np.ascontiguousarray(
        sf[2].T.reshape(4, 128, 10).transpose(1, 0, 2)
    ).astype(BF16_NP)

    bb = np.zeros((128, 16), np.float32)
    t0 = thr(conv_gammas[0], conv_betas[0], conv_means[0], conv_vars[0])
    bb[0:64, 0], bb[64:128, 0] = t0, t0
    t1b = thr(conv_gammas[1], conv_betas[1], conv_means[1], conv_vars[1])
    bb[0:64, 1], bb[64:128, 1] = t1b, t1b
    bb[:, 2] = thr(conv_gammas[2], conv_betas[2], conv_means[2], conv_vars[2])
    bb[:, 3] = thr(conv_gammas[3], conv_betas[3], conv_means[3], conv_vars[3])
    t4 = thr(conv_gammas[4], conv_betas[4], conv_means[4], conv_vars[4])
    bb[:, 4], bb[:, 5] = t4[0:128], t4[128:256]
    t5 = thr(conv_gammas[5], conv_betas[5], conv_means[5], conv_vars[5])
    bb[:, 6], bb[:, 7] = t5[0:128], t5[128:256]
    t1 = thr(fc_gammas[0], fc_betas[0], fc_means[0], fc_vars[0])
    t2 = thr(fc_gammas[1], fc_betas[1], fc_means[1], fc_vars[1])
    for g in range(4):
        bb[:, 8 + g] = t1[128 * g : 128 * g + 128]
        bb[:, 12 + g] = t2[128 * g : 128 * g + 128]
    d["bb"] = bb
    return d


def make_xcol(xc, N):
    """Host im2col: xc [N,3,36,36] -> [nq, 4, 27, qsz, 34, 36] fp32."""
    G = 4
    npg = N // G
    qsz = max(1, npg // 4)
    nq = npg // qsz
    xf = np.asarray(xc, np.float32).reshape(N, 3, 1296)
    xp = np.concatenate([xf, np.zeros((N, 3, 38), np.float32)], axis=2)
    out = np.empty((nq, G, 27, qsz, 35, 36), np.float32)
    for ky in range(3):
        for kx in range(3):
            for ci in range(3):
                p = (ky * 3 + kx) * 3 + ci
                off = ky * 36 + kx
                v = xp[:, ci, off : off + 1260].reshape(nq, G, qsz, 35, 36)
                out[:, :, p] = v
    return out


_CACHE = {}


def _get_nc(N):
    if N not in _CACHE:
        nc = build_program(N)
        nc.compile()
        _CACHE[N] = nc
    return _CACHE[N]


LAST_RESULTS = None


def kernel(**inputs):
    global LAST_RESULTS
    from concourse.bass_utils import run_bass_kernel_spmd

    x = np.asarray(inputs["x"], np.float32)
    B = x.shape[0]
    N = B // N_CORES
    shared = prep_shared(
        inputs["conv_ws"], inputs["conv_gammas"], inputs["conv_betas"],
        inputs["conv_means"], inputs["conv_vars"], inputs["fc_ws"],
        inputs["fc_gammas"], inputs["fc_betas"], inputs["fc_means"],
        inputs["fc_vars"],
    )
    in_maps = []
    for c in range(N_CORES):
        m = dict(shared)
        m["xcol"] = make_xcol(x[c * N : (c + 1) * N], N)
        in_maps.append(m)

    nc = _get_nc(N)
    res = run_bass_kernel_spmd(
        nc, in_maps, core_ids=list(range(N_CORES)),
        trace=bool(int(os.environ.get("KERNEL_TRACE", "0"))),
    )
    LAST_RESULTS = res

    logits = np.concatenate([res.results[c]["out"].T for c in range(N_CORES)], axis=0)
    # final BN (affine=False) + log_softmax on host, fp32 to match reference
    m2 = np.asarray(inputs["fc_means"][2], np.float32)
    v2 = np.asarray(inputs["fc_vars"][2], np.float32)
    z = ((logits - m2) / np.sqrt(v2 + np.float32(EPS))).astype(np.float32)
    zm = z - z.max(axis=1, keepdims=True)
    z = zm - np.log(np.exp(zm).sum(axis=1, keepdims=True, dtype=np.float32))
    return z.astype(np.float32)


# revision 14
# speedup vs baseline: 1.0004x; 1.0004x over previous
"""Trainium2 Bass kernel for nn_CNV_Cifar10_Binary (binary CNN, CIFAR-like).

Strategy (pure data parallel, 8 cores x 64 images):
 - All binarized convs/FCs run as bf16 matmuls (values are exactly +-1 -> exact).
 - Layer 0 (real-valued input) runs in fp32 with 4-way PE row-tiling (K=27 via
   host-side im2col; partition groups at base 0/32/64/96).
 - sign(BN(x)) == sign(x + c) since inv>0; applied by ScalarE Sign activation
   with per-channel bias straight out of PSUM; maxpool commutes with sign and
   is done after sign by VectorE max ops on bf16.
 - Cin=64 layers (L1, L2) pack ky in the contraction dim: partitions 64..127
   hold the same activations shifted by one row (built with one SBUF->SBUF DMA),
   so taps (ky=0,1) fuse into K=128 matmuls; ky=2 uses a K=64 matmul.
 - Final FC3 output (10 logits) is copied out raw; affine-free BN + log_softmax
   run on host (exact, negligible work).
"""

import os
import sys

import numpy as np

sys.path.insert(0, "/opt/trn_rl_repo")

import concourse.bass as bass  # noqa: E402
import concourse.bacc as bacc  # noqa: E402
import concourse.mybir as mybir  # noqa: E402
import concourse.tile as tile  # noqa: E402

F32 = mybir.dt.float32
BF16 = mybir.dt.bfloat16
AF = mybir.ActivationFunctionType
ALU = mybir.AluOpType
BF16_NP = mybir.dt.np(BF16)

N_CORES = 8
EPS = 1e-5


def build_program(N):
    """Build the per-core Bass program for N images."""
    assert N % 8 == 0
    G = 4                      # PE row-tile groups for layer 0
    npg = N // G               # images per group
    qsz = max(1, npg // 4)     # images per group per X-load block
    nq = npg // qsz            # number of X-load blocks ("quarters")
    assert nq % 2 == 0
    qph = nq // 2              # quarters per half
    NH = N // 2                # images per half (S1 buffer covers a half)

    nc = bacc.Bacc("TRN2", target_bir_lowering=False, debug=False)

    # ---- DRAM I/O ----
    xcol = nc.dram_tensor("xcol", [nq, G, 27, qsz, 35, 36], F32, kind="ExternalInput")
    w0d = nc.dram_tensor("w0", [27, 64], F32, kind="ExternalInput")
    w1ad = nc.dram_tensor("w1a", [128, 3, 64], BF16, kind="ExternalInput")
    w1bd = nc.dram_tensor("w1b", [64, 3, 64], BF16, kind="ExternalInput")
    w2ad = nc.dram_tensor("w2a", [128, 3, 128], BF16, kind="ExternalInput")
    w2bd = nc.dram_tensor("w2b", [64, 3, 128], BF16, kind="ExternalInput")
    w3d = nc.dram_tensor("w3", [128, 9, 128], BF16, kind="ExternalInput")
    w4d = nc.dram_tensor("w4", [128, 2, 9, 128], BF16, kind="ExternalInput")
    w5d = nc.dram_tensor("w5", [2, 128, 2, 9, 128], BF16, kind="ExternalInput")
    fw1d = nc.dram_tensor("fw1", [128, 2, 512], BF16, kind="ExternalInput")
    fw2d = nc.dram_tensor("fw2", [128, 4, 512], BF16, kind="ExternalInput")
    fw3d = nc.dram_tensor("fw3", [128, 4, 10], BF16, kind="ExternalInput")
    bbd = nc.dram_tensor("bb", [128, 16], F32, kind="ExternalInput")
    outd = nc.dram_tensor("out", [10, N], F32, kind="ExternalOutput")

    with tile.TileContext(nc) as tc:
        with (
            tc.tile_pool(name="wpool", bufs=1) as wp,
            tc.tile_pool(name="xpool", bufs=2) as xp,
            tc.tile_pool(name="apool", bufs=1) as ap,
            tc.tile_pool(name="spool", bufs=3) as sp,
            tc.tile_pool(name="pspool", bufs=2, space="PSUM") as pp,
            tc.tile_pool(name="pspool2", bufs=3, space="PSUM") as pp2,
            tc.tile_pool(name="dpool", bufs=1, space="DRAM") as dp,
        ):
            # ---- weights / consts ----
            W0 = wp.tile([128, 64], F32, tag="w0")
            for g in range(G):
                nc.sync.dma_start(W0[32 * g : 32 * g + 27, :], w0d[:, :])
            W1A = wp.tile([128, 3, 64], BF16, tag="w1a")
            nc.sync.dma_start(W1A[:, :, :], w1ad[:, :, :])
            W1B = wp.tile([64, 3, 64], BF16, tag="w1b")
            nc.sync.dma_start(W1B[:, :, :], w1bd[:, :, :])
            W2A = wp.tile([128, 3, 128], BF16, tag="w2a")
            nc.sync.dma_start(W2A[:, :, :], w2ad[:, :, :])
            W2B = wp.tile([64, 3, 128], BF16, tag="w2b")
            nc.sync.dma_start(W2B[:, :, :], w2bd[:, :, :])
            W3 = wp.tile([128, 9, 128], BF16, tag="w3")
            nc.sync.dma_start(W3[:, :, :], w3d[:, :, :])
            W4 = wp.tile([128, 2, 9, 128], BF16, tag="w4")
            nc.sync.dma_start(W4[:, :, :, :], w4d[:, :, :, :])
            W5 = [wp.tile([128, 2, 9, 128], BF16, tag=f"w5{i}", name=f"W5_{i}") for i in range(2)]
            for i in range(2):
                nc.sync.dma_start(W5[i][:, :, :, :], w5d[i, :, :, :, :])
            FW1 = wp.tile([128, 2, 512], BF16, tag="fw1")
            nc.sync.dma_start(FW1[:, :, :], fw1d[:, :, :])
            FW2 = wp.tile([128, 4, 512], BF16, tag="fw2")
            nc.sync.dma_start(FW2[:, :, :], fw2d[:, :, :])
            FW3 = wp.tile([128, 4, 10], BF16, tag="fw3")
            nc.sync.dma_start(FW3[:, :, :], fw3d[:, :, :])
            BB = wp.tile([128, 16], F32, tag="bb")
            nc.sync.dma_start(BB[:, :], bbd[:, :])

            # ---- persistent activation buffers ----
            S2 = ap.tile([128, N, 16, 16], BF16, tag="s2")
            S2pre = ap.tile([128, N // 2, 16, 16], BF16, tag="s2p")
            S4 = ap.tile([128, N, 6, 6], BF16, tag="s4")
            S5 = [ap.tile([128, N, 4, 4], BF16, tag=f"s5{i}", name=f"S5_{i}") for i in range(2)]
            S6 = [ap.tile([128, N], BF16, tag=f"s6{i}", name=f"S6_{i}") for i in range(2)]
            F1 = [ap.tile([128, N], BF16, tag=f"f1{i}", name=f"F1_{i}") for i in range(4)]
            F2 = [ap.tile([128, N], BF16, tag=f"f2{i}", name=f"F2_{i}") for i in range(4)]
            OB = ap.tile([10, N], F32, tag="ob")

            # ================= L0 + L1 (blocked in image halves) ==========
            # L0: 4-way PE row tiling (K=27) x 2-way col tiling. Column group 1
            # recomputes the same rows shifted +1 so PSUM partitions 64..127
            # hold the ky=1-shifted copy -> a single 128-lane Sign evac writes
            # both the activations and their shifted duplicate into S1.
            D2 = dp.tile([128, N // 2, 16, 16], BF16, tag="d2")
            for h in range(2):
                S1 = ap.tile([128, NH, 34, 34], BF16, tag="s1", name=f"S1_{h}")
                for qq in range(qph):
                    q = h * qph + qq
                    X = xp.tile([128, qsz, 35, 36], F32, tag="x", name=f"X_{q}")
                    for g in range(G):
                        nc.sync.dma_start(
                            X[32 * g : 32 * g + 27, :, :, :], xcol[q, g, :, :, :, :]
                        )
                    for k in range(qsz):
                        for g in range(G):
                            n_img = q * G * qsz + g * qsz + k
                            nl = n_img - h * NH
                            W0g = W0[32 * g : 32 * g + 27, :]
                            Xg = X[32 * g : 32 * g + 27, k]
                            P = pp2.tile([128, 2, 512], F32, tag="ps2")
                            for c, y0 in enumerate((0, 15)):
                                nc.tensor.matmul(
                                    P[0:64, c, 0:510], W0g, Xg[:, y0 : y0 + 15, 0:34],
                                    start=True, stop=True, skip_group_check=True, tile_position=(32 * g, 0))
                                nc.tensor.matmul(
                                    P[64:128, c, 0:510], W0g, Xg[:, y0 + 1 : y0 + 16, 0:34],
                                    start=True, stop=True, skip_group_check=True, tile_position=(32 * g, 64))
                            nc.scalar.activation(
                                S1[:, nl, 0:30, :], P[:, :, 0:510],
                                AF.Sign, bias=BB[:, 0:1])
                            Pt = pp.tile([128, 512], F32, tag="ps")
                            nc.tensor.matmul(
                                Pt[0:64, 0:136], W0g, Xg[:, 30:34, 0:34],
                                start=True, stop=True, skip_group_check=True, tile_position=(32 * g, 0))
                            nc.tensor.matmul(
                                Pt[64:128, 0:136], W0g, Xg[:, 31:35, 0:34],
                                start=True, stop=True, skip_group_check=True, tile_position=(32 * g, 64))
                            nc.scalar.activation(
                                S1[:, nl, 30:34, :], Pt[:, 0:136],
                                AF.Sign, bias=BB[:, 0:1])

                # ---- L1: 64->64, 32x32 out, pool to 16x16 ----
                # col-tile over image pairs: psum parts 0..63 <- img nA,
                # parts 64..127 <- img nB; 128-lane sign + pool into S2pre.
                for nl in range(NH // 2):
                    nA, nB = nl, nl + NH // 2
                    pidx = h * (NH // 2) + nl
                    for y0 in (0, 16):
                        pq = pp if y0 == 0 else pp2
                        P = pq.tile([128, 512], F32, tag="ps" if y0 == 0 else "ps2", name=f"P1_{y0}")
                        for kx in range(3):
                            nc.tensor.matmul(
                                P[0:64, :], W1A[:, kx, :],
                                S1[0:128, nA, y0 : y0 + 16, kx : kx + 32],
                                start=(kx == 0), stop=False, skip_group_check=True, tile_position=(0, 0))
                            nc.tensor.matmul(
                                P[64:128, :], W1A[:, kx, :],
                                S1[0:128, nB, y0 : y0 + 16, kx : kx + 32],
                                start=(kx == 0), stop=False, skip_group_check=True, tile_position=(0, 64))
                        for kx in range(3):
                            nc.tensor.matmul(
                                P[0:64, :], W1B[0:64, kx, :],
                                S1[0:64, nA, y0 + 2 : y0 + 18, kx : kx + 32],
                                start=False, stop=(kx == 2), skip_group_check=True, tile_position=(0, 0))
                            nc.tensor.matmul(
                                P[64:128, :], W1B[0:64, kx, :],
                                S1[0:64, nB, y0 + 2 : y0 + 18, kx : kx + 32],
                                start=False, stop=(kx == 2), skip_group_check=True, tile_position=(0, 64))
                        T = sp.tile([128, 16, 16, 2], BF16, tag="t1")
                        nc.scalar.activation(
                            T[:, :, :, :], P[:, :], AF.Sign, bias=BB[:, 1:2]
                        )
                        U = sp.tile([128, 8, 2, 16], BF16, tag="u1")
                        nc.vector.tensor_tensor(
                            U[:, :, :, :], T[:, :, :, 0], T[:, :, :, 1], op=ALU.max
                        )
                        nc.vector.tensor_tensor(
                            S2pre[:, pidx, y0 // 2 : y0 // 2 + 8, :],
                            U[:, :, 0, :], U[:, :, 1, :], op=ALU.max,
                        )
                nc.sync.dma_start(
                    D2[:, h * (NH // 2) : (h + 1) * (NH // 2)],
                    S2pre[:, h * (NH // 2) : (h + 1) * (NH // 2)],
                )
            # re-layout S2pre (img-pair partitions) -> S2 (ky-dup partitions)
            # via DRAM bounce (SBUF->SBUF DMA unsupported).
            NH2 = N // 4
            for h in range(2):
                pa, pb = NH2 * 2 * h, NH2 * 2 * h + NH2
                nc.sync.dma_start(S2[0:64, pa : pa + NH2], D2[0:64, h * NH2 : (h + 1) * NH2])
                nc.sync.dma_start(S2[0:64, pb : pb + NH2], D2[64:128, h * NH2 : (h + 1) * NH2])
                nc.sync.dma_start(S2[64:128, pa : pa + NH2, 0:15, :],
                                  D2[0:64, h * NH2 : (h + 1) * NH2, 1:16, :])
                nc.sync.dma_start(S2[64:128, pb : pb + NH2, 0:15, :],
                                  D2[64:128, h * NH2 : (h + 1) * NH2, 1:16, :])
            # ================= L2: 64->128, 16x16 -> 14x14 ================
            S3 = ap.tile([128, N, 14, 14], BF16, tag="s1")  # reuse S1 slot
            for n0 in range(0, N, 2):
                pq, tg = (pp, "ps") if (n0 // 2) % 2 == 0 else (pp2, "ps2")
                P = pq.tile([128, 2, 14, 14], F32, tag=tg, name=f"P2_{n0}")
                for kx in range(3):
                    nc.tensor.matmul(
                        P[:, :, :, :],
                        W2A[:, kx, :],
                        S2[0:128, n0 : n0 + 2, 0:14, kx : kx + 14],
                        start=(kx == 0),
                        stop=False,
                    )
                for kx in range(3):
                    nc.tensor.matmul(
                        P[:, :, :, :],
                        W2B[0:64, kx, :],
                        S2[0:64, n0 : n0 + 2, 2:16, kx : kx + 14],
                        start=False,
                        stop=(kx == 2),
                    )
                nc.scalar.activation(
                    S3[:, n0 : n0 + 2, :, :], P[:, :, :, :], AF.Sign, bias=BB[:, 2:3]
                )

            # ================= L3: 128->128, 14->12, pool to 6x6 ==========
            n0 = 0
            ci3 = 0
            while n0 < N:
                nn = min(3, N - n0)
                pq, tg = (pp, "ps") if ci3 % 2 == 0 else (pp2, "ps2")
                ci3 += 1
                P = pq.tile([128, 3, 12, 12], F32, tag=tg, name=f"P3_{n0}")
                t = 0
                for ky in range(3):
                    for kx in range(3):
                        nc.tensor.matmul(
                            P[:, :nn, :, :],
                            W3[:, t, :],
                            S3[:, n0 : n0 + nn, ky : ky + 12, kx : kx + 12],
                            start=(t == 0),
                            stop=(t == 8),
                        )
                        t += 1
                T = sp.tile([128, 3, 12, 6, 2], BF16, tag="t3")
                nc.scalar.activation(
                    T[:, :nn, :, :, :], P[:, :nn, :, :], AF.Sign, bias=BB[:, 3:4]
                )
                U = sp.tile([128, 3, 6, 2, 6], BF16, tag="u3")
                nc.vector.tensor_tensor(
                    U[:, :nn, :, :, :], T[:, :nn, :, :, 0], T[:, :nn, :, :, 1], op=ALU.max
                )
                nc.vector.tensor_tensor(
                    S4[:, n0 : n0 + nn, :, :], U[:, :nn, :, 0, :], U[:, :nn, :, 1, :],
                    op=ALU.max,
                )
                n0 += nn

            # ================= L4: 128->256, 6->4 =========================
            for n0 in range(0, N, 32):
                nn = min(32, N - n0)
                for cg in range(2):
                    pq, tg = (pp, "ps") if cg == 0 else (pp2, "ps2")
                    P = pq.tile([128, 32, 4, 4], F32, tag=tg, name=f"P4_{cg}")
                    t = 0
                    for ky in range(3):
                        for kx in range(3):
                            nc.tensor.matmul(
                                P[:, :nn, :, :],
                                W4[:, cg, t, :],
                                S4[:, n0 : n0 + nn, ky : ky + 4, kx : kx + 4],
                                start=(t == 0),
                                stop=(t == 8),
                            )
                            t += 1
                    nc.scalar.activation(
                        S5[cg][:, n0 : n0 + nn, :, :],
                        P[:, :nn, :, :],
                        AF.Sign,
                        bias=BB[:, 4 + cg : 5 + cg],
                    )

            # ================= L5: 256->256, 4->2, pool to 1 ==============
            for cg in range(2):
                P = pp.tile([128, N, 2, 2], F32, tag="ps")
                first = True
                for ci in range(2):
                    t = 0
                    for ky in range(3):
                        for kx in range(3):
                            nc.tensor.matmul(
                                P[:, :, :, :],
                                W5[ci][:, cg, t, :],
                                S5[ci][:, :, ky : ky + 2, kx : kx + 2],
                                start=first,
                                stop=(ci == 1 and t == 8),
                            )
                            first = False
                            t += 1
                T = sp.tile([128, N, 2, 2], BF16, tag="t5")
                nc.scalar.activation(
                    T[:, :, :, :], P[:, :, :, :], AF.Sign, bias=BB[:, 6 + cg : 7 + cg]
                )
                U = sp.tile([128, N, 2], BF16, tag="u5")
                nc.vector.tensor_tensor(U[:, :, :], T[:, :, :, 0], T[:, :, :, 1], op=ALU.max)
                nc.vector.tensor_tensor(S6[cg][:, :], U[:, :, 0], U[:, :, 1], op=ALU.max)

            # ================= FC1/FC2/FC3 ================================
            for g in range(4):
                P = pp.tile([128, N], F32, tag="ps")
                for kc in range(2):
                    nc.tensor.matmul(
                        P[:, :], FW1[:, kc, 128 * g : 128 * g + 128], S6[kc][:, :],
                        start=(kc == 0), stop=(kc == 1),
                    )
                nc.scalar.activation(F1[g][:, :], P[:, :], AF.Sign, bias=BB[:, 8 + g : 9 + g])
            for g in range(4):
                P = pp.tile([128, N], F32, tag="ps")
                for kc in range(4):
                    nc.tensor.matmul(
                        P[:, :], FW2[:, kc, 128 * g : 128 * g + 128], F1[kc][:, :],
                        start=(kc == 0), stop=(kc == 3),
                    )
                nc.scalar.activation(F2[g][:, :], P[:, :], AF.Sign, bias=BB[:, 12 + g : 13 + g])
            P = pp.tile([10, N], F32, tag="ps")
            for kc in range(4):
                nc.tensor.matmul(
                    P[:, :], FW3[:, kc, :], F2[kc][:, :],
                    start=(kc == 0), stop=(kc == 3),
                )
            nc.vector.tensor_copy(OB[:, :], P[:, :])
            nc.sync.dma_start(outd[:, :], OB[:, :])

    return nc


def prep_shared(conv_ws, conv_gammas, conv_betas, conv_means, conv_vars,
                fc_ws, fc_gammas, fc_betas, fc_means, fc_vars):
    """Host-side: fold BN into per-channel sign thresholds; lay out weights."""
    s = [np.sign(np.asarray(w, np.float32)) for w in conv_ws]
    sf = [np.sign(np.asarray(w, np.float32)) for w in fc_ws]

    def thr(g, b, m, v):
        inv = np.asarray(g) / np.sqrt(np.asarray(v) + EPS)
        assert np.all(inv > 0)
        return (np.asarray(b) / inv - np.asarray(m)).astype(np.float32)

    d = {}
    d["w0"] = s[0].transpose(2, 3, 1, 0).reshape(27, 64).astype(np.float32)
    for i, nm in ((1, "w1"), (2, "w2")):
        a = s[i].transpose(2, 1, 3, 0)  # (ky, ci, kx, co)
        d[nm + "a"] = np.ascontiguousarray(a[0:2].reshape(128, 3, -1)).astype(BF16_NP)
        d[nm + "b"] = np.ascontiguousarray(a[2]).astype(BF16_NP)
    d["w3"] = np.ascontiguousarray(
        s[3].transpose(1, 2, 3, 0).reshape(128, 9, 128)
    ).astype(BF16_NP)
    d["w4"] = np.ascontiguousarray(
        s[4].reshape(2, 128, 128, 3, 3).transpose(2, 0, 3, 4, 1).reshape(128, 2, 9, 128)
    ).astype(BF16_NP)
    d["w5"] = np.ascontiguousarray(
        s[5].reshape(2, 128, 2, 128, 3, 3).transpose(2, 3, 0, 4, 5, 1)
        .reshape(2, 128, 2, 9, 128)
    ).astype(BF16_NP)
    d["fw1"] = np.ascontiguousarray(
        sf[0].T.reshape(2, 128, 512).transpose(1, 0, 2)
    ).astype(BF16_NP)
    d["fw2"] = np.ascontiguousarray(
        sf[1].T.reshape(4, 128, 512).transpose(1, 0, 2)
    ).astype(BF16_NP)
    d["fw3"] = np.ascontiguousarray(
        sf[2].T.reshape(4, 128, 10).transpose(1, 0, 2)
    ).astype(BF16_NP)

    bb = np.zeros((128, 16), np.float32)
    t0 = thr(conv_gammas[0], conv_betas[0], conv_means[0], conv_vars[0])
    bb[0:64, 0], bb[64:128, 0] = t0, t0
    t1b = thr(conv_gammas[1], conv_betas[1], conv_means[1], conv_vars[1])
    bb[0:64, 1], bb[64:128, 1] = t1b, t1b
    bb[:, 2] = thr(conv_gammas[2], conv_betas[2], conv_means[2], conv_vars[2])
    bb[:, 3] = thr(conv_gammas[3], conv_betas[3], conv_means[3], conv_vars[3])
    t4 = thr(conv_gammas[4], conv_betas[4], conv_means[4], conv_vars[4])
    bb[:, 4], bb[:, 5] = t4[0:128], t4[128:256]
    t5 = thr(conv_gammas[5], conv_betas[5], conv_means[5], conv_vars[5])
    bb[:, 6], bb[:, 7] = t5[0:128], t5[128:256]
    t1 = thr(fc_gammas[0], fc_betas[0], fc_means[0], fc_vars[0])
    t2 = thr(fc_gammas[1], fc_betas[1], fc_means[1], fc_vars[1])
    for g in range(4):
        bb[:, 8 + g] = t1[128 * g : 128 * g + 128]
        bb[:, 12 + g] = t2[128 * g : 128 * g + 128]
    d["bb"] = bb
    return d


def make_xcol(xc, N):
    """Host im2col: xc [N,3,36,36] -> [nq, 4, 27, qsz, 34, 36] fp32."""
    G = 4
    npg = N // G
    qsz = max(1, npg // 4)
    nq = npg // qsz
    xf = np.asarray(xc, np.float32).reshape(N, 3, 1296)
    xp = np.concatenate([xf, np.zeros((N, 3, 38), np.float32)], axis=2)
    out = np.empty((nq, G, 27, qsz, 35, 36), np.float32)
    for ky in range(3):
        for kx in range(3):
            for ci in range(3):
                p = (ky * 3 + kx) * 3 + ci
                off = ky * 36 + kx
                v = xp[:, ci, off : off + 1260].reshape(nq, G, qsz, 35, 36)
                out[:, :, p] = v
    return out


_CACHE = {}


def _get_nc(N):
    if N not in _CACHE:
        nc = build_program(N)
        nc.compile()
        _CACHE[N] = nc
    return _CACHE[N]


LAST_RESULTS = None


def kernel(**inputs):
    global LAST_RESULTS
    from concourse.bass_utils import run_bass_kernel_spmd

    x = np.asarray(inputs["x"], np.float32)
    B = x.shape[0]
    N = B // N_CORES
    shared = prep_shared(
        inputs["conv_ws"], inputs["conv_gammas"], inputs["conv_betas"],
        inputs["conv_means"], inputs["conv_vars"], inputs["fc_ws"],
        inputs["fc_gammas"], inputs["fc_betas"], inputs["fc_means"],
        inputs["fc_vars"],
    )
    in_maps = []
    for c in range(N_CORES):
        m = dict(shared)
        m["xcol"] = make_xcol(x[c * N : (c + 1) * N], N)
        in_maps.append(m)

    nc = _get_nc(N)
    res = run_bass_kernel_spmd(
        nc, in_maps, core_ids=list(range(N_CORES)),
        trace=bool(int(os.environ.get("KERNEL_TRACE", "0"))),
    )
    LAST_RESULTS = res

    logits = np.concatenate([res.results[c]["out"].T for c in range(N_CORES)], axis=0)
    # final BN (affine=False) + log_softmax on host, fp32 to match reference
    m2 = np.asarray(inputs["fc_means"][2], np.float32)
    v2 = np.asarray(inputs["fc_vars"][2], np.float32)
    z = ((logits - m2) / np.sqrt(v2 + np.float32(EPS))).astype(np.float32)
    zm = z - z.max(axis=1, keepdims=True)
    z = zm - np.log(np.exp(zm).sum(axis=1, keepdims=True, dtype=np.float32))
    return z.astype(np.float32)


# revision 15
# speedup vs baseline: 1.0058x; 1.0054x over previous
"""Trainium2 Bass kernel for nn_CNV_Cifar10_Binary (binary CNN, CIFAR-like).

Strategy (pure data parallel, 8 cores x 64 images):
 - All binarized convs/FCs run as bf16 matmuls (values are exactly +-1 -> exact).
 - Layer 0 (real-valued input) runs in fp32 with 4-way PE row-tiling (K=27 via
   host-side im2col; partition groups at base 0/32/64/96).
 - sign(BN(x)) == sign(x + c) since inv>0; applied by ScalarE Sign activation
   with per-channel bias straight out of PSUM; maxpool commutes with sign and
   is done after sign by VectorE max ops on bf16.
 - Cin=64 layers (L1, L2) pack ky in the contraction dim: partitions 64..127
   hold the same activations shifted by one row (built with one SBUF->SBUF DMA),
   so taps (ky=0,1) fuse into K=128 matmuls; ky=2 uses a K=64 matmul.
 - Final FC3 output (10 logits) is copied out raw; affine-free BN + log_softmax
   run on host (exact, negligible work).
"""

import os
import sys

import numpy as np

sys.path.insert(0, "/opt/trn_rl_repo")

import concourse.bass as bass  # noqa: E402
import concourse.bacc as bacc  # noqa: E402
import concourse.mybir as mybir  # noqa: E402
import concourse.tile as tile  # noqa: E402

F32 = mybir.dt.float32
BF16 = mybir.dt.bfloat16
AF = mybir.ActivationFunctionType
ALU = mybir.AluOpType
BF16_NP = mybir.dt.np(BF16)

N_CORES = 8
EPS = 1e-5


def build_program(N):
    """Build the per-core Bass program for N images."""
    assert N % 8 == 0
    G = 4                      # PE row-tile groups for layer 0
    npg = N // G               # images per group
    qsz = max(1, npg // 4)     # images per group per X-load block
    nq = npg // qsz            # number of X-load blocks ("quarters")
    assert nq % 2 == 0
    qph = nq // 2              # quarters per half
    NH = N // 2                # images per half (S1 buffer covers a half)

    nc = bacc.Bacc("TRN2", target_bir_lowering=False, debug=False)

    # ---- DRAM I/O ----
    xcol = nc.dram_tensor("xcol", [nq, G, 27, qsz, 35, 36], F32, kind="ExternalInput")
    w0d = nc.dram_tensor("w0", [27, 64], F32, kind="ExternalInput")
    w1ad = nc.dram_tensor("w1a", [128, 3, 64], BF16, kind="ExternalInput")
    w1bd = nc.dram_tensor("w1b", [64, 3, 64], BF16, kind="ExternalInput")
    w2ad = nc.dram_tensor("w2a", [128, 3, 128], BF16, kind="ExternalInput")
    w2bd = nc.dram_tensor("w2b", [64, 3, 128], BF16, kind="ExternalInput")
    w3d = nc.dram_tensor("w3", [128, 9, 128], BF16, kind="ExternalInput")
    w4d = nc.dram_tensor("w4", [128, 2, 9, 128], BF16, kind="ExternalInput")
    w5d = nc.dram_tensor("w5", [2, 128, 2, 9, 128], BF16, kind="ExternalInput")
    fw1d = nc.dram_tensor("fw1", [128, 2, 512], BF16, kind="ExternalInput")
    fw2d = nc.dram_tensor("fw2", [128, 4, 512], BF16, kind="ExternalInput")
    fw3d = nc.dram_tensor("fw3", [128, 4, 10], BF16, kind="ExternalInput")
    bbd = nc.dram_tensor("bb", [128, 16], F32, kind="ExternalInput")
    outd = nc.dram_tensor("out", [10, N], F32, kind="ExternalOutput")

    with tile.TileContext(nc) as tc:
        with (
            tc.tile_pool(name="wpool", bufs=1) as wp,
            tc.tile_pool(name="xpool", bufs=2) as xp,
            tc.tile_pool(name="apool", bufs=1) as ap,
            tc.tile_pool(name="spool", bufs=3) as sp,
            tc.tile_pool(name="pspool", bufs=2, space="PSUM") as pp,
            tc.tile_pool(name="pspool2", bufs=3, space="PSUM") as pp2,
            tc.tile_pool(name="dpool", bufs=1, space="DRAM") as dp,
        ):
            # ---- weights / consts ----
            W0 = wp.tile([128, 64], F32, tag="w0")
            for g in range(G):
                nc.sync.dma_start(W0[32 * g : 32 * g + 27, :], w0d[:, :])
            W1A = wp.tile([128, 3, 64], BF16, tag="w1a")
            nc.sync.dma_start(W1A[:, :, :], w1ad[:, :, :])
            W1B = wp.tile([64, 3, 64], BF16, tag="w1b")
            nc.sync.dma_start(W1B[:, :, :], w1bd[:, :, :])
            W2A = wp.tile([128, 3, 128], BF16, tag="w2a")
            nc.sync.dma_start(W2A[:, :, :], w2ad[:, :, :])
            W2B = wp.tile([64, 3, 128], BF16, tag="w2b")
            nc.sync.dma_start(W2B[:, :, :], w2bd[:, :, :])
            W3 = wp.tile([128, 9, 128], BF16, tag="w3")
            nc.sync.dma_start(W3[:, :, :], w3d[:, :, :])
            W4 = wp.tile([128, 2, 9, 128], BF16, tag="w4")
            nc.sync.dma_start(W4[:, :, :, :], w4d[:, :, :, :])
            W5 = [wp.tile([128, 2, 9, 128], BF16, tag=f"w5{i}", name=f"W5_{i}") for i in range(2)]
            for i in range(2):
                nc.sync.dma_start(W5[i][:, :, :, :], w5d[i, :, :, :, :])
            FW1 = wp.tile([128, 2, 512], BF16, tag="fw1")
            nc.sync.dma_start(FW1[:, :, :], fw1d[:, :, :])
            FW2 = wp.tile([128, 4, 512], BF16, tag="fw2")
            nc.sync.dma_start(FW2[:, :, :], fw2d[:, :, :])
            FW3 = wp.tile([128, 4, 10], BF16, tag="fw3")
            nc.sync.dma_start(FW3[:, :, :], fw3d[:, :, :])
            BB = wp.tile([128, 16], F32, tag="bb")
            nc.sync.dma_start(BB[:, :], bbd[:, :])

            # ---- persistent activation buffers ----
            S2 = ap.tile([128, N, 16, 16], BF16, tag="s2")
            S2pre = ap.tile([128, N // 2, 16, 16], BF16, tag="s2p")
            S4 = ap.tile([128, N, 6, 6], BF16, tag="s4")
            S5 = [ap.tile([128, N, 4, 4], BF16, tag=f"s5{i}", name=f"S5_{i}") for i in range(2)]
            S6 = [ap.tile([128, N], BF16, tag=f"s6{i}", name=f"S6_{i}") for i in range(2)]
            F1 = [ap.tile([128, N], BF16, tag=f"f1{i}", name=f"F1_{i}") for i in range(4)]
            F2 = [ap.tile([128, N], BF16, tag=f"f2{i}", name=f"F2_{i}") for i in range(4)]
            OB = ap.tile([10, N], F32, tag="ob")

            # ================= L0 + L1 (blocked in image halves) ==========
            # L0: 4-way PE row tiling (K=27) x 2-way col tiling. Column group 1
            # recomputes the same rows shifted +1 so PSUM partitions 64..127
            # hold the ky=1-shifted copy -> a single 128-lane Sign evac writes
            # both the activations and their shifted duplicate into S1.
            for h in range(2):
                S1 = ap.tile([128, NH, 34, 34], BF16, tag="s1", name=f"S1_{h}")
                for qq in range(qph):
                    q = h * qph + qq
                    X = xp.tile([128, qsz, 35, 36], F32, tag="x", name=f"X_{q}")
                    for g in range(G):
                        nc.sync.dma_start(
                            X[32 * g : 32 * g + 27, :, :, :], xcol[q, g, :, :, :, :]
                        )
                    for k in range(qsz):
                        for g in range(G):
                            n_img = q * G * qsz + g * qsz + k
                            nl = n_img - h * NH
                            W0g = W0[32 * g : 32 * g + 27, :]
                            Xg = X[32 * g : 32 * g + 27, k]
                            P = pp2.tile([128, 2, 512], F32, tag="ps2")
                            for c, y0 in enumerate((0, 15)):
                                nc.tensor.matmul(
                                    P[0:64, c, 0:510], W0g, Xg[:, y0 : y0 + 15, 0:34],
                                    start=True, stop=True, skip_group_check=True, tile_position=(32 * g, 0))
                                nc.tensor.matmul(
                                    P[64:128, c, 0:510], W0g, Xg[:, y0 + 1 : y0 + 16, 0:34],
                                    start=True, stop=True, skip_group_check=True, tile_position=(32 * g, 64))
                            nc.scalar.activation(
                                S1[:, nl, 0:30, :], P[:, :, 0:510],
                                AF.Sign, bias=BB[:, 0:1])
                            Pt = pp.tile([128, 512], F32, tag="ps")
                            nc.tensor.matmul(
                                Pt[0:64, 0:136], W0g, Xg[:, 30:34, 0:34],
                                start=True, stop=True, skip_group_check=True, tile_position=(32 * g, 0))
                            nc.tensor.matmul(
                                Pt[64:128, 0:136], W0g, Xg[:, 31:35, 0:34],
                                start=True, stop=True, skip_group_check=True, tile_position=(32 * g, 64))
                            nc.scalar.activation(
                                S1[:, nl, 30:34, :], Pt[:, 0:136],
                                AF.Sign, bias=BB[:, 0:1])

                # ---- L1: 64->64, 32x32 out, pool to 16x16 ----
                # col-tile over image pairs: psum parts 0..63 <- img nA,
                # parts 64..127 <- img nB; 128-lane sign + pool into S2pre.
                for nl in range(NH // 2):
                    nA, nB = nl, nl + NH // 2
                    pidx = h * (NH // 2) + nl
                    for y0 in (0, 16):
                        pq = pp if y0 == 0 else pp2
                        P = pq.tile([128, 512], F32, tag="ps" if y0 == 0 else "ps2", name=f"P1_{y0}")
                        for kx in range(3):
                            nc.tensor.matmul(
                                P[0:64, :], W1A[:, kx, :],
                                S1[0:128, nA, y0 : y0 + 16, kx : kx + 32],
                                start=(kx == 0), stop=False, skip_group_check=True, tile_position=(0, 0))
                            nc.tensor.matmul(
                                P[64:128, :], W1A[:, kx, :],
                                S1[0:128, nB, y0 : y0 + 16, kx : kx + 32],
                                start=(kx == 0), stop=False, skip_group_check=True, tile_position=(0, 64))
                        for kx in range(3):
                            nc.tensor.matmul(
                                P[0:64, :], W1B[0:64, kx, :],
                                S1[0:64, nA, y0 + 2 : y0 + 18, kx : kx + 32],
                                start=False, stop=(kx == 2), skip_group_check=True, tile_position=(0, 0))
                            nc.tensor.matmul(
                                P[64:128, :], W1B[0:64, kx, :],
                                S1[0:64, nB, y0 + 2 : y0 + 18, kx : kx + 32],
                                start=False, stop=(kx == 2), skip_group_check=True, tile_position=(0, 64))
                        T = sp.tile([128, 16, 16, 2], BF16, tag="t1")
                        nc.scalar.activation(
                            T[:, :, :, :], P[:, :], AF.Sign, bias=BB[:, 1:2]
                        )
                        U = sp.tile([128, 8, 2, 16], BF16, tag="u1")
                        nc.vector.tensor_tensor(
                            U[:, :, :, :], T[:, :, :, 0], T[:, :, :, 1], op=ALU.max
                        )
                        nc.vector.tensor_tensor(
                            S2pre[:, pidx, y0 // 2 : y0 // 2 + 8, :],
                            U[:, :, 0, :], U[:, :, 1, :], op=ALU.max,
                        )
                # re-layout S2pre (img-pair partitions) -> S2 (ky-dup partitions)
                pa, pb = NH * h, NH * h + NH // 2
                hs = slice(h * (NH // 2), (h + 1) * (NH // 2))
                nc.sync.dma_start(S2[0:64, pa : pa + NH // 2], S2pre[0:64, hs])
                nc.sync.dma_start(S2[0:64, pb : pb + NH // 2], S2pre[64:128, hs])
                nc.sync.dma_start(S2[64:128, pa : pa + NH // 2, 0:15, :],
                                  S2pre[0:64, hs, 1:16, :])
                nc.sync.dma_start(S2[64:128, pb : pb + NH // 2, 0:15, :],
                                  S2pre[64:128, hs, 1:16, :])
            # ================= L2: 64->128, 16x16 -> 14x14 ================
            S3 = ap.tile([128, N, 14, 14], BF16, tag="s1")  # reuse S1 slot
            for n0 in range(0, N, 2):
                pq, tg = (pp, "ps") if (n0 // 2) % 2 == 0 else (pp2, "ps2")
                P = pq.tile([128, 2, 14, 14], F32, tag=tg, name=f"P2_{n0}")
                for kx in range(3):
                    nc.tensor.matmul(
                        P[:, :, :, :],
                        W2A[:, kx, :],
                        S2[0:128, n0 : n0 + 2, 0:14, kx : kx + 14],
                        start=(kx == 0),
                        stop=False,
                    )
                for kx in range(3):
                    nc.tensor.matmul(
                        P[:, :, :, :],
                        W2B[0:64, kx, :],
                        S2[0:64, n0 : n0 + 2, 2:16, kx : kx + 14],
                        start=False,
                        stop=(kx == 2),
                    )
                nc.scalar.activation(
                    S3[:, n0 : n0 + 2, :, :], P[:, :, :, :], AF.Sign, bias=BB[:, 2:3]
                )

            # ================= L3: 128->128, 14->12, pool to 6x6 ==========
            n0 = 0
            ci3 = 0
            while n0 < N:
                nn = min(3, N - n0)
                pq, tg = (pp, "ps") if ci3 % 2 == 0 else (pp2, "ps2")
                ci3 += 1
                P = pq.tile([128, 3, 12, 12], F32, tag=tg, name=f"P3_{n0}")
                t = 0
                for ky in range(3):
                    for kx in range(3):
                        nc.tensor.matmul(
                            P[:, :nn, :, :],
                            W3[:, t, :],
                            S3[:, n0 : n0 + nn, ky : ky + 12, kx : kx + 12],
                            start=(t == 0),
                            stop=(t == 8),
                        )
                        t += 1
                T = sp.tile([128, 3, 12, 6, 2], BF16, tag="t3")
                nc.scalar.activation(
                    T[:, :nn, :, :, :], P[:, :nn, :, :], AF.Sign, bias=BB[:, 3:4]
                )
                U = sp.tile([128, 3, 6, 2, 6], BF16, tag="u3")
                nc.vector.tensor_tensor(
                    U[:, :nn, :, :, :], T[:, :nn, :, :, 0], T[:, :nn, :, :, 1], op=ALU.max
                )
                nc.vector.tensor_tensor(
                    S4[:, n0 : n0 + nn, :, :], U[:, :nn, :, 0, :], U[:, :nn, :, 1, :],
                    op=ALU.max,
                )
                n0 += nn

            # ================= L4: 128->256, 6->4 =========================
            for n0 in range(0, N, 32):
                nn = min(32, N - n0)
                for cg in range(2):
                    pq, tg = (pp, "ps") if cg == 0 else (pp2, "ps2")
                    P = pq.tile([128, 32, 4, 4], F32, tag=tg, name=f"P4_{cg}")
                    t = 0
                    for ky in range(3):
                        for kx in range(3):
                            nc.tensor.matmul(
                                P[:, :nn, :, :],
                                W4[:, cg, t, :],
                                S4[:, n0 : n0 + nn, ky : ky + 4, kx : kx + 4],
                                start=(t == 0),
                                stop=(t == 8),
                            )
                            t += 1
                    nc.scalar.activation(
                        S5[cg][:, n0 : n0 + nn, :, :],
                        P[:, :nn, :, :],
                        AF.Sign,
                        bias=BB[:, 4 + cg : 5 + cg],
                    )

            # ================= L5: 256->256, 4->2, pool to 1 ==============
            for cg in range(2):
                P = pp.tile([128, N, 2, 2], F32, tag="ps")
                first = True
                for ci in range(2):
                    t = 0
                    for ky in range(3):
                        for kx in range(3):
                            nc.tensor.matmul(
                                P[:, :, :, :],
                                W5[ci][:, cg, t, :],
                                S5[ci][:, :, ky : ky + 2, kx : kx + 2],
                                start=first,
                                stop=(ci == 1 and t == 8),
                            )
                            first = False
                            t += 1
                T = sp.tile([128, N, 2, 2], BF16, tag="t5")
                nc.scalar.activation(
                    T[:, :, :, :], P[:, :, :, :], AF.Sign, bias=BB[:, 6 + cg : 7 + cg]
                )
                U = sp.tile([128, N, 2], BF16, tag="u5")
                nc.vector.tensor_tensor(U[:, :, :], T[:, :, :, 0], T[:, :, :, 1], op=ALU.max)
                nc.vector.tensor_tensor(S6[cg][:, :], U[:, :, 0], U[:, :, 1], op=ALU.max)

            # ================= FC1/FC2/FC3 ================================
            for g in range(4):
                P = pp.tile([128, N], F32, tag="ps")
                for kc in range(2):
                    nc.tensor.matmul(
                        P[:, :], FW1[:, kc, 128 * g : 128 * g + 128], S6[kc][:, :],
                        start=(kc == 0), stop=(kc == 1),
                    )
                nc.scalar.activation(F1[g][:, :], P[:, :], AF.Sign, bias=BB[:, 8 + g : 9 + g])
            for g in range(4):
                P = pp.tile([128, N], F32, tag="ps")
                for kc in range(4):
                    nc.tensor.matmul(
                        P[:, :], FW2[:, kc, 128 * g : 128 * g + 128], F1[kc][:, :],
                        start=(kc == 0), stop=(kc == 3),
                    )
                nc.scalar.activation(F2[g][:, :], P[:, :], AF.Sign, bias=BB[:, 12 + g : 13 + g])
            P = pp.tile([10, N], F32, tag="ps")
            for kc in range(4):
                nc.tensor.matmul(
                    P[:, :], FW3[:, kc, :], F2[kc][:, :],
                    start=(kc == 0), stop=(kc == 3),
                )
            nc.vector.tensor_copy(OB[:, :], P[:, :])
            nc.sync.dma_start(outd[:, :], OB[:, :])

    return nc


def prep_shared(conv_ws, conv_gammas, conv_betas, conv_means, conv_vars,
                fc_ws, fc_gammas, fc_betas, fc_means, fc_vars):
    """Host-side: fold BN into per-channel sign thresholds; lay out weights."""
    s = [np.sign(np.asarray(w, np.float32)) for w in conv_ws]
    sf = [np.sign(np.asarray(w, np.float32)) for w in fc_ws]

    def thr(g, b, m, v):
        inv = np.asarray(g) / np.sqrt(np.asarray(v) + EPS)
        assert np.all(inv > 0)
        return (np.asarray(b) / inv - np.asarray(m)).astype(np.float32)

    d = {}
    d["w0"] = s[0].transpose(2, 3, 1, 0).reshape(27, 64).astype(np.float32)
    for i, nm in ((1, "w1"), (2, "w2")):
        a = s[i].transpose(2, 1, 3, 0)  # (ky, ci, kx, co)
        d[nm + "a"] = np.ascontiguousarray(a[0:2].reshape(128, 3, -1)).astype(BF16_NP)
        d[nm + "b"] = np.ascontiguousarray(a[2]).astype(BF16_NP)
    d["w3"] = np.ascontiguousarray(
        s[3].transpose(1, 2, 3, 0).reshape(128, 9, 128)
    ).astype(BF16_NP)
    d["w4"] = np.ascontiguousarray(
        s[4].reshape(2, 128, 128, 3, 3).transpose(2, 0, 3, 4, 1).reshape(128, 2, 9, 128)
    ).astype(BF16_NP)
    d["w5"] = np.ascontiguousarray(
        s[5].reshape(2, 128, 2, 128, 3, 3).transpose(2, 3, 0, 4, 5, 1)
        .reshape(2, 128, 2, 9, 128)
    ).astype(BF16_NP)
    d["fw1"] = np.ascontiguousarray(
        sf[0].T.reshape(2, 128, 512).transpose(1, 0, 2)
    ).astype(BF16_NP)
    d["fw2"] = np.ascontiguousarray(
        sf[1].T.reshape(4, 128, 512).transpose(1, 0, 2)
    ).astype(BF16_NP)
    d["fw3"] = np.ascontiguousarray(
        sf[2].T.reshape(4, 128, 10).transpose(1, 0, 2)
    ).astype(BF16_NP)

    bb = np.zeros((128, 16), np.float32)
    t0 = thr(conv_gammas[0], conv_betas[0], conv_means[0], conv_vars[0])
    bb[0:64, 0], bb[64:128, 0] = t0, t0
    t1b = thr(conv_gammas[1], conv_betas[1], conv_means[1], conv_vars[1])
    bb[0:64, 1], bb[64:128, 1] = t1b, t1b
    bb[:, 2] = thr(conv_gammas[2], conv_betas[2], conv_means[2], conv_vars[2])
    bb[:, 3] = thr(conv_gammas[3], conv_betas[3], conv_means[3], conv_vars[3])
    t4 = thr(conv_gammas[4], conv_betas[4], conv_means[4], conv_vars[4])
    bb[:, 4], bb[:, 5] = t4[0:128], t4[128:256]
    t5 = thr(conv_gammas[5], conv_betas[5], conv_means[5], conv_vars[5])
    bb[:, 6], bb[:, 7] = t5[0:128], t5[128:256]
    t1 = thr(fc_gammas[0], fc_betas[0], fc_means[0], fc_vars[0])
    t2 = thr(fc_gammas[1], fc_betas[1], fc_means[1], fc_vars[1])
    for g in range(4):
        bb[:, 8 + g] = t1[128 * g : 128 * g + 128]
        bb[:, 12 + g] = t2[128 * g : 128 * g + 128]
    d["bb"] = bb
    return d


def make_xcol(xc, N):
    """Host im2col: xc [N,3,36,36] -> [nq, 4, 27, qsz, 34, 36] fp32."""
    G = 4
    npg = N // G
    qsz = max(1, npg // 4)
    nq = npg // qsz
    xf = np.asarray(xc, np.float32).reshape(N, 3, 1296)
    xp = np.concatenate([xf, np.zeros((N, 3, 38), np.float32)], axis=2)
    out = np.empty((nq, G, 27, qsz, 35, 36), np.float32)
    for ky in range(3):
        for kx in range(3):
            for ci in range(3):
                p = (ky * 3 + kx) * 3 + ci
                off = ky * 36 + kx
                v = xp[:, ci, off : off + 1260].reshape(nq, G, qsz, 35, 36)
                out[:, :, p] = v
    return out


_CACHE = {}


def _get_nc(N):
    if N not in _CACHE:
        nc = build_program(N)
        nc.compile()
        _CACHE[N] = nc
    return _CACHE[N]


LAST_RESULTS = None


def kernel(**inputs):
    global LAST_RESULTS
    from concourse.bass_utils import run_bass_kernel_spmd

    x = np.asarray(inputs["x"], np.float32)
    B = x.shape[0]
    N = B // N_CORES
    shared = prep_shared(
        inputs["conv_ws"], inputs["conv_gammas"], inputs["conv_betas"],
        inputs["conv_means"], inputs["conv_vars"], inputs["fc_ws"],
        inputs["fc_gammas"], inputs["fc_betas"], inputs["fc_means"],
        inputs["fc_vars"],
    )
    in_maps = []
    for c in range(N_CORES):
        m = dict(shared)
        m["xcol"] = make_xcol(x[c * N : (c + 1) * N], N)
        in_maps.append(m)

    nc = _get_nc(N)
    res = run_bass_kernel_spmd(
        nc, in_maps, core_ids=list(range(N_CORES)),
        trace=bool(int(os.environ.get("KERNEL_TRACE", "0"))),
    )
    LAST_RESULTS = res

    logits = np.concatenate([res.results[c]["out"].T for c in range(N_CORES)], axis=0)
    # final BN (affine=False) + log_softmax on host, fp32 to match reference
    m2 = np.asarray(inputs["fc_means"][2], np.float32)
    v2 = np.asarray(inputs["fc_vars"][2], np.float32)
    z = ((logits - m2) / np.sqrt(v2 + np.float32(EPS))).astype(np.float32)
    zm = z - z.max(axis=1, keepdims=True)
    z = zm - np.log(np.exp(zm).sum(axis=1, keepdims=True, dtype=np.float32))
    return z.astype(np.float32)
